# revision 19
# baseline (speedup 1.0000x reference)
"""DGCNN forward on 8 NeuronCores (Bass/Tile, TRN2).

Sharding: graphs are independent -> 400 graphs split 50/core; the small
Linear/Conv weights are replicated. Each graph has exactly 128 nodes =
one SBUF partition span, so message passing is a dense per-graph matmul
against (A+I)^T, built on-device by a CCE-add indirect DMA scatter of
host-prepared (cell index, count) pairs (integer preprocessing only).
All float math runs on-device in fp32: the sortpool top-k ordering is
sensitive to ~1e-8 perturbations, so no bf16/fp32r on the value path.

Only native walrus-supported instructions are used (no Q7 extended ISA),
and a BIR post-pass splits Tile's multi-wait instructions into NoOp+wait
chains (this toolchain's walrus accepts one sync-wait per instruction).
"""

import sys

import numpy as np

sys.path.insert(0, "/opt/trn_rl_repo")

import concourse.bass as bass  # noqa: E402
import concourse.bass_utils as bass_utils  # noqa: E402
import concourse.mybir as mybir  # noqa: E402
import concourse.tile as tile  # noqa: E402
from concourse.bass_utils import run_bass_kernel_spmd  # noqa: E402


def _split_multiwaits(bir_json):
    import json

    m = json.loads(bir_json)
    cnt = 0
    for f in m.get("functions", []):
        for blk in f.get("blocks", []):
            out = []
            for ins in blk.get("instructions", []):
                si = ins.get("sync_info")
                ow = (si or {}).get("on_wait") or []
                if len(ow) > 1:
                    for w in ow[:-1]:
                        cnt += 1
                        out.append(
                            {
                                "name": f"wsplit-{cnt}",
                                "opcode": "NoOp",
                                "engine": ins["engine"],
                                "ins": [],
                                "outs": [],
                                "debug": ins.get("debug", 0),
                                "sync_info": {"on_wait": [w], "on_update": []},
                            }
                        )
                    si["on_wait"] = [ow[-1]]
                out.append(ins)
            blk["instructions"] = out
    return json.dumps(m).encode()


_orig_compile_bir_kernel = bass_utils.compile_bir_kernel


def _patched_compile_bir_kernel(bir_json, tmpdir, neff_name="file.neff"):
    return _orig_compile_bir_kernel(_split_multiwaits(bir_json), tmpdir, neff_name)


if bass_utils.compile_bir_kernel is _orig_compile_bir_kernel:
    bass_utils.compile_bir_kernel = _patched_compile_bir_kernel
    import concourse.bass2jax as _b2j

    if getattr(_b2j, "compile_bir_kernel", None) is _orig_compile_bir_kernel:
        _b2j.compile_bir_kernel = _patched_compile_bir_kernel

# problem constants
G = 400
NPG = 128
F0 = 128
LAT = [32, 32, 32, 1]
D = 97
K = 30
C1, C2, KW2 = 16, 32, 5
TCONV = K // 2 - KW2 + 1  # 11
OUTW = C2 * TCONV  # 352

NCORES = 8
GPC = G // NCORES  # 50
NCORE = GPC * NPG  # 6400
NCELL = NPG * NCORE  # 819200 cells in (A+I)^T per core
FIN = [F0] + LAT[:-1]
FOUT = LAT
OFF = [0, 32, 64, 96]

f32 = mybir.dt.float32
i32 = mybir.dt.int32
Alu = mybir.AluOpType
Act = mybir.ActivationFunctionType


def _batches(total, bs):
    out = []
    s = 0
    while s < total:
        out.append((s, min(bs, total - s)))
        s += bs
    return out


def _prep_edges(edge_src, edge_dst):
    """Index-format conversion: COO edge list -> dense per-core (A+I)^T
    count matrices [128 src, 50*128 (graph,dst)] (pure integer graph
    structure; no feature/weight data involved)."""
    es = np.asarray(edge_src).astype(np.int64)
    ed = np.asarray(edge_dst).astype(np.int64)
    g = ed >> 7
    s = es & 127
    d = ed & 127
    ati = np.zeros((G, NPG, NPG), np.float32)  # [g, s, d] = (A+I)^T blocks
    np.add.at(ati, (g, s, d), 1.0)
    ati += np.eye(NPG, dtype=np.float32)[None]
    # per core: [128 s, 50 g * 128 d]
    ati = ati.reshape(NCORES, GPC, NPG, NPG).transpose(0, 2, 1, 3)
    return np.ascontiguousarray(ati.reshape(NCORES, NPG, NCORE))


def _build(nc, debug=False):
    # ---- DRAM I/O ----
    x_d = nc.dram_tensor("x", [NCORE, F0], f32, kind="ExternalInput")
    degs_d = nc.dram_tensor("degs", [GPC, NPG], f32, kind="ExternalInput")
    w_d = [
        nc.dram_tensor(f"w{i}", [FIN[i], FOUT[i]], f32, kind="ExternalInput")
        for i in range(4)
    ]
    bcat_d = nc.dram_tensor("bcat", [1, D], f32, kind="ExternalInput")
    c1w_d = nc.dram_tensor("c1w", [C1, D], f32, kind="ExternalInput")
    c1b_d = nc.dram_tensor("c1b", [C1, 1], f32, kind="ExternalInput")
    c2w_d = nc.dram_tensor("c2w", [C2, C1 * KW2], f32, kind="ExternalInput")
    c2b_d = nc.dram_tensor("c2b", [C2, 1], f32, kind="ExternalInput")
    ident_d = nc.dram_tensor("ident", [NPG, NPG], f32, kind="ExternalInput")
    iota1_d = nc.dram_tensor("iota1", [NPG, 1], f32, kind="ExternalInput")
    iota50_d = nc.dram_tensor("iota50", [GPC, NPG], f32, kind="ExternalInput")
    ones1_d = nc.dram_tensor("ones1", [1, NPG], f32, kind="ExternalInput")
    ati_d = nc.dram_tensor("ati", [NPG, NCORE], f32, kind="ExternalInput")
    out_d = nc.dram_tensor("out_feat", [GPC, OUTW], f32, kind="ExternalOutput")
    oidx_d = nc.dram_tensor("out_idx", [GPC, K], i32, kind="ExternalOutput")
    idxscr_d = nc.dram_tensor("idx_scratch", [GPC, K], f32, kind="Internal")
    if debug:
        dbg_ati_d = nc.dram_tensor("dbg_ati", [NPG, NCORE], f32, kind="ExternalOutput")
        dbg_cat_d = nc.dram_tensor("dbg_cat", [NPG, GPC * D], f32, kind="ExternalOutput")
        dbg_sv_d = nc.dram_tensor("dbg_sv", [GPC, NPG], f32, kind="ExternalOutput")
        dbg_sel_d = nc.dram_tensor("dbg_sel", [NPG, GPC * K], f32, kind="ExternalOutput")
        dbg_pool_d = nc.dram_tensor("dbg_pool", [D, GPC * K], f32, kind="ExternalOutput")

    from contextlib import ExitStack

    with tile.TileContext(nc) as tc, ExitStack() as ctx:
        pp = ctx.enter_context(tc.tile_pool(name="persist", bufs=1))
        pb = ctx.enter_context(tc.tile_pool(name="batch", bufs=3))
        ps = ctx.enter_context(tc.tile_pool(name="psA", bufs=3, space="PSUM"))
        ps1 = ctx.enter_context(tc.tile_pool(name="psB", bufs=3, space="PSUM"))
        ps2 = ctx.enter_context(tc.tile_pool(name="psC", bufs=2, space="PSUM"))

        # ---- constants / weights ----
        id128 = pp.tile([NPG, NPG], f32)
        nc.sync.dma_start(id128[:], ident_d[:])
        iota_f = pp.tile([NPG, 1], f32)
        nc.sync.dma_start(iota_f[:], iota1_d[:])
        iota50 = pp.tile([GPC, NPG], f32)
        nc.sync.dma_start(iota50[:], iota50_d[:])
        ones1 = pp.tile([1, NPG], f32)
        nc.sync.dma_start(ones1[:], ones1_d[:])

        w_sb = []
        for i in range(4):
            w = pp.tile([FIN[i], FOUT[i]], f32, name=f"w{i}sb")
            nc.sync.dma_start(w[:], w_d[i][:])
            w_sb.append(w)
        bcat_row = pp.tile([1, D], f32)
        nc.sync.dma_start(bcat_row[:], bcat_d[:])
        c1w_sb = pp.tile([C1, D], f32)
        nc.sync.dma_start(c1w_sb[:], c1w_d[:])
        c1b_sb = pp.tile([C1, 1], f32)
        nc.sync.dma_start(c1b_sb[:], c1b_d[:])
        c2w_sb = pp.tile([C2, C1 * KW2], f32)
        nc.sync.dma_start(c2w_sb[:], c2w_d[:])
        c2b_sb = pp.tile([C2, 1], f32)
        nc.sync.dma_start(c2b_sb[:], c2b_d[:])

        # bias row replicated to 128 partitions via K=1 matmul with ones
        brp = ps2.tile([NPG, D], f32, name="brp", tag="c")
        nc.tensor.matmul(brp[:], ones1[:], bcat_row[:], start=True, stop=True)
        brep = pp.tile([NPG, D], f32)
        nc.vector.tensor_copy(brep[:], brp[:])

        # degrees [50,128] -> [128,50] -> reciprocal
        degsT = pp.tile([GPC, NPG], f32)
        nc.sync.dma_start(degsT[:], degs_d[:])
        dps = ps2.tile([NPG, GPC], f32, name="dps", tag="c")
        nc.tensor.transpose(dps[:], degsT[:], id128[0:GPC, 0:GPC])
        rdeg = pp.tile([NPG, GPC], f32)
        nc.vector.reciprocal(rdeg[:], dps[:])

        # conv1 weight transposed [97,16]
        c1ps = ps2.tile([D, C1], f32, name="c1ps", tag="c")
        nc.tensor.transpose(c1ps[:], c1w_sb[:], id128[0:C1, 0:C1])
        c1wT = pp.tile([D, C1], f32)
        nc.vector.tensor_copy(c1wT[:], c1ps[:])

        # conv2 weights: 5 slices [16,32]
        w2T = pp.tile([C1, KW2 * C2], f32)
        for h in range(KW2):
            sl = c2w_sb[:].rearrange("p (c h) -> p c h", h=KW2)[:, :, h : h + 1]
            sl = sl.rearrange("p c o -> p (c o)")
            wps = ps2.tile([C1, C2], f32, name="wps", tag="c")
            nc.tensor.transpose(wps[:], sl, id128[0:C2, 0:C2])
            nc.vector.tensor_copy(w2T[:, h * C2 : (h + 1) * C2], wps[:])

        # ---- (A+I)^T: dense count matrix from host ----
        ati = pp.tile([NPG, NCORE], f32)
        nc.sync.dma_start(ati[:], ati_d[:])
        if debug:
            nc.sync.dma_start(dbg_ati_d[:], ati[:])

        # ---- load X node-major, transpose per graph in place -> X^T ----
        x_sb = pp.tile([NPG, NCORE], f32)
        nc.sync.dma_start(
            x_sb[:].rearrange("p (g f) -> p g f", g=GPC),
            x_d[:].rearrange("(g p) f -> p g f", p=NPG),
        )
        xt = x_sb
        for q0, qn in _batches(GPC, 4):
            xtp = ps.tile([NPG, 4 * NPG], f32, name="xtp", tag="a")
            for j in range(qn):
                gg = q0 + j
                nc.tensor.transpose(
                    xtp[:, j * NPG : (j + 1) * NPG],
                    x_sb[:, gg * NPG : (gg + 1) * NPG],
                    id128[:],
                )
            nc.vector.tensor_copy(
                xt[:, q0 * NPG : (q0 + qn) * NPG], xtp[:, : qn * NPG]
            )

        # ---- layers ----
        curA = pp.tile([32, GPC * NPG], f32)
        curB = pp.tile([32, GPC * NPG], f32)
        cat = pp.tile([NPG, GPC * D], f32)
        rdeg3 = rdeg[:].rearrange("p (g o) -> p g o", o=1)

        state = None
        for l in range(4):
            fi, fo, off = FIN[l], FOUT[l], OFF[l]
            nxt = curA if (l % 2 == 0) else curB
            brep3 = (
                brep[:, off : off + fo]
                .rearrange("p (o c) -> p o c", o=1)
            )
            for b0, bn in _batches(GPC, 8):
                hps = ps.tile([NPG, 8 * 32], f32, name="hps", tag="a")
                for j in range(bn):
                    gg = b0 + j
                    if l == 0:
                        lhsT = xt[:, gg * NPG : (gg + 1) * NPG]
                    else:
                        lhsT = state[:, gg * NPG : (gg + 1) * NPG]
                    nc.tensor.matmul(
                        hps[:, j * fo : (j + 1) * fo], lhsT, w_sb[l][:],
                        start=True, stop=True,
                    )
                hsb = pb.tile([NPG, 8 * 32], f32, name="hsb", tag="hsb")
                nc.vector.tensor_copy(hsb[:, : bn * fo], hps[:, : bn * fo])
                lps = ps1.tile([NPG, 8 * 32], f32, name="lps", tag="b")
                for j in range(bn):
                    gg = b0 + j
                    nc.tensor.matmul(
                        lps[:, j * fo : (j + 1) * fo],
                        ati[:, gg * NPG : (gg + 1) * NPG],
                        hsb[:, j * fo : (j + 1) * fo],
                        start=True, stop=True,
                    )
                # tanh((LIN + b) * 1/deg) -> cat (node-major)
                t1 = pb.tile([NPG, 8 * 32], f32, name="t1", tag="t1")
                t13 = t1[:, : bn * fo].rearrange("p (g c) -> p g c", g=bn)
                lps3 = lps[:, : bn * fo].rearrange("p (g c) -> p g c", g=bn)
                nc.vector.tensor_tensor(
                    out=t13, in0=lps3,
                    in1=brep3.to_broadcast([NPG, bn, fo]),
                    op=Alu.add,
                )
                nc.vector.tensor_tensor(
                    out=t13, in0=t13,
                    in1=rdeg3[:, b0 : b0 + bn, :].to_broadcast([NPG, bn, fo]),
                    op=Alu.mult,
                )
                cat3 = cat[:].rearrange("p (g c) -> p g c", g=GPC)[
                    :, b0 : b0 + bn, off : off + fo
                ]
                nc.scalar.activation(cat3, t13, Act.Tanh)
            if l < 3:
                for gbase, gn in _batches(GPC, 4):
                    tps = ps1.tile([32, 4 * NPG], f32, name="tps", tag="b")
                    for j in range(gn):
                        gg = gbase + j
                        src = cat[:, gg * D + off : gg * D + off + fo]
                        nc.tensor.transpose(
                            tps[:, j * NPG : (j + 1) * NPG], src, id128[:]
                        )
                    nc.vector.tensor_copy(
                        nxt[:, gbase * NPG : (gbase + gn) * NPG],
                        tps[:, : gn * NPG],
                    )
                state = nxt

        # ---- sortpool: per-graph top-30 on last channel ----
        sv_ap = cat[:].rearrange("p (g c) -> p g c", c=D)[:, :, D - 1 : D]
        sv_ap = sv_ap.rearrange("p g o -> p (g o)")
        svps = ps2.tile([GPC, NPG], f32, name="svps", tag="c")
        nc.tensor.transpose(svps[:], sv_ap, id128[:])
        svT = pp.tile([GPC, NPG], f32)
        nc.vector.tensor_copy(svT[:], svps[:])
        if debug:
            nc.sync.dma_start(dbg_cat_d[:], cat[:])
            nc.sync.dma_start(dbg_sv_d[:], svT[:])

        idx = pp.tile([GPC, K], f32)
        mred = pp.tile([GPC, 1], f32)
        teq = pp.tile([GPC, NPG], f32)
        tt = pp.tile([GPC, NPG], f32)
        for k in range(K):
            nc.vector.tensor_reduce(
                out=mred[:], in_=svT[:], axis=mybir.AxisListType.X, op=Alu.max
            )
            nc.vector.tensor_scalar(
                out=teq[:], in0=svT[:], scalar1=mred[:], scalar2=None,
                op0=Alu.is_equal,
            )
            nc.vector.scalar_tensor_tensor(
                out=tt[:], in0=teq[:], scalar=-16384.0, in1=iota50[:],
                op0=Alu.mult, op1=Alu.add,
            )
            nc.vector.tensor_reduce(
                out=mred[:], in_=tt[:], axis=mybir.AxisListType.X, op=Alu.min
            )
            nc.vector.tensor_scalar(
                out=idx[:, k : k + 1], in0=mred[:], scalar1=16384.0, scalar2=None,
                op0=Alu.add,
            )
            if k < K - 1:
                nc.vector.tensor_scalar(
                    out=teq[:], in0=iota50[:], scalar1=idx[:, k : k + 1],
                    scalar2=None, op0=Alu.is_equal,
                )
                nc.vector.scalar_tensor_tensor(
                    out=svT[:], in0=teq[:], scalar=-1e30, in1=svT[:],
                    op0=Alu.mult, op1=Alu.add,
                )
        idxi = pp.tile([GPC, K], i32)
        nc.vector.tensor_copy(idxi[:], idx[:])
        nc.sync.dma_start(oidx_d[:], idxi[:])

        # broadcast indices to all partitions (DRAM roundtrip + ones-matmul)
        w_idx = nc.sync.dma_start(idxscr_d[:], idx[:])
        idxrow = pp.tile([1, GPC * K], f32)
        r_idx = nc.sync.dma_start(
            idxrow[:], idxscr_d[:].rearrange("(o g) k -> o (g k)", o=1)
        )
        tile.add_dep_helper(r_idx.ins, w_idx.ins, reason="idx roundtrip")
        idxrep = pp.tile([NPG, GPC * K], f32)
        for c0, cn in _batches(GPC * K, 512):
            irp = ps2.tile([NPG, 512], f32, name="irp", tag="c")
            nc.tensor.matmul(
                irp[:, :cn], ones1[:], idxrow[:, c0 : c0 + cn],
                start=True, stop=True,
            )
            nc.vector.tensor_copy(idxrep[:, c0 : c0 + cn], irp[:, :cn])
        sel = pp.tile([NPG, GPC * K], f32)
        nc.vector.tensor_scalar(
            out=sel[:], in0=idxrep[:], scalar1=iota_f[:], scalar2=None,
            op0=Alu.is_equal,
        )
        if debug:
            nc.sync.dma_start(dbg_sel_d[:], sel[:])

        # pooled^T [97,30] per graph = cat_g^T @ sel_g  (exact one-hot select)
        pooledT = pp.tile([D, GPC * K], f32)
        for b0, bn in _batches(GPC, 4):
            pps = ps.tile([D, 4 * K], f32, name="pps", tag="a")
            for j in range(bn):
                gg = b0 + j
                nc.tensor.matmul(
                    pps[:, j * K : (j + 1) * K],
                    cat[:, gg * D : (gg + 1) * D],
                    sel[:, gg * K : (gg + 1) * K],
                    start=True, stop=True,
                )
            nc.vector.tensor_copy(
                pooledT[:, b0 * K : (b0 + bn) * K], pps[:, : bn * K]
            )

        if debug:
            nc.sync.dma_start(dbg_pool_d[:], pooledT[:])
        # conv1 + relu
        c1r = pp.tile([C1, GPC * K], f32)
        for b0, bn in _batches(GPC, 4):
            cps = ps1.tile([C1, 4 * K], f32, name="cps", tag="b")
            for j in range(bn):
                gg = b0 + j
                nc.tensor.matmul(
                    cps[:, j * K : (j + 1) * K],
                    c1wT[:],
                    pooledT[:, gg * K : (gg + 1) * K],
                    start=True, stop=True,
                )
            nc.scalar.activation(
                c1r[:, b0 * K : (b0 + bn) * K], cps[:, : bn * K],
                Act.Relu, bias=c1b_sb[:],
            )

        # maxpool pairs
        p1 = pp.tile([C1, GPC * (K // 2)], f32)
        c1r4 = c1r[:].rearrange("p (g k two) -> p g k two", g=GPC, two=2)
        p14 = p1[:].rearrange("p (g k) -> p g k", g=GPC).rearrange(
            "p g (k o) -> p g k o", o=1
        )
        nc.vector.tensor_tensor(
            out=p14, in0=c1r4[:, :, :, 0:1], in1=c1r4[:, :, :, 1:2], op=Alu.max
        )

        # conv2 (5-tap) + relu
        outsb = pp.tile([C2, GPC * TCONV], f32)
        p13 = p1[:].rearrange("p (g k) -> p g k", g=GPC)
        for b0, bn in _batches(GPC, 8):
            ops = ps2.tile([C2, 8 * TCONV], f32, name="ops", tag="c")
            for h in range(KW2):
                nc.tensor.matmul(
                    ops[:, : bn * TCONV],
                    w2T[:, h * C2 : (h + 1) * C2],
                    p13[:, b0 : b0 + bn, h : h + TCONV],
                    start=(h == 0), stop=(h == KW2 - 1),
                )
            nc.scalar.activation(
                outsb[:, b0 * TCONV : (b0 + bn) * TCONV],
                ops[:, : bn * TCONV],
                Act.Relu, bias=c2b_sb[:],
            )

        nc.sync.dma_start(
            out_d[:].rearrange("g (o t) -> o g t", o=C2),
            outsb[:].rearrange("p (g t) -> p g t", g=GPC),
        )
    return nc


_CACHE = {}


def _get_nc():
    if "nc" not in _CACHE:
        nc = bass.Bass()
        _build(nc)
        _CACHE["nc"] = nc
    return _CACHE["nc"]


def _run(inputs, trace=False):
    node_feat = np.ascontiguousarray(np.asarray(inputs["node_feat"], dtype=np.float32))
    node_degs = np.ascontiguousarray(
        np.asarray(inputs["node_degs"], dtype=np.float32).reshape(G, NPG)
    )
    ati_np = _prep_edges(inputs["edge_src"], inputs["edge_dst"])
    bcat = np.concatenate(
        [np.asarray(inputs[f"b{i}"], dtype=np.float32).ravel() for i in range(4)]
    )[None, :]
    c1w = np.asarray(inputs["conv1_w"], dtype=np.float32)
    c1b = np.asarray(inputs["conv1_b"], dtype=np.float32).reshape(C1, 1)
    c2w = np.asarray(inputs["conv2_w"], dtype=np.float32).reshape(C2, C1 * KW2)
    c2b = np.asarray(inputs["conv2_b"], dtype=np.float32).reshape(C2, 1)
    ident = np.eye(NPG, dtype=np.float32)
    iota1 = np.arange(NPG, dtype=np.float32)[:, None]
    iota50 = np.tile(np.arange(NPG, dtype=np.float32), (GPC, 1))
    ones1 = np.ones((1, NPG), np.float32)

    nc = _get_nc()
    in_maps = []
    for c in range(NCORES):
        m = {
            "x": np.ascontiguousarray(node_feat[c * NCORE : (c + 1) * NCORE]),
            "degs": np.ascontiguousarray(node_degs[c * GPC : (c + 1) * GPC]),
            "bcat": bcat,
            "c1w": c1w,
            "c1b": c1b,
            "c2w": c2w,
            "c2b": c2b,
            "ident": ident,
            "iota1": iota1,
            "iota50": iota50,
            "ones1": ones1,
            "ati": ati_np[c],
        }
        for i in range(4):
            m[f"w{i}"] = np.asarray(inputs[f"W{i}"], dtype=np.float32)
        in_maps.append(m)
    res = run_bass_kernel_spmd(
        nc, in_maps, core_ids=list(range(NCORES)), trace=trace
    )
    out = np.concatenate([r["out_feat"] for r in res.results], axis=0)
    idx = np.concatenate([r["out_idx"] for r in res.results], axis=0)
    return out, idx, res


def kernel(**inputs):
    out, idx, _ = _run(inputs)
    return out, idx
